# revision 8
# baseline (speedup 1.0000x reference)
"""GNN message passing (scatter-add of gathered node features) on 8 TRN2 NeuronCores.

Strategy (edge + node hybrid sharding, no collectives):
  - Outputs are node-sharded: core k owns destination rows [k*12500, (k+1)*12500).
  - Edges are assigned to the core owning their destination row.
  - Per core, each edge is one "token": gather x[col] (one 256B row) from HBM via
    dma_gather into an SBUF message buffer, then accumulate into the core's output
    shard in HBM via dma_scatter_add (SDMA CCE read-modify-write add descriptors).
  - dma_gather indices are int16, so x is addressed in 4 segments of 25000 rows;
    tokens inside each block are grouped by source segment (<=4 sub-gathers/block).
  - Duplicate-destination correctness: concurrent CCE RMW descriptors to the same
    row race (hardware-verified), and per-engine ring order does NOT serialize the
    read-modify-write. Therefore destination rows are UNIQUE within each scatter
    call (each row's edges are dealt to distinct blocks on the host) and scatter
    calls are serialized by waiting each scatter's completion semaphore before
    issuing the next. Rows with more edges than there are main blocks spill into
    extra cleanup blocks. Gathers run pipelined ahead on a separate SWDGE queue.
"""

import numpy as np

# ---- problem constants (hardcoded; must match the harness inputs) ----
N_NODES = 100000
N_EDGES = 1250000
D = 64
NCORES = 8

DEFAULT_PARAMS = dict(
    n_nodes=N_NODES,
    d=D,
    ncores=8,
    shard=12500,      # destination rows per core  (ncores*shard == n_nodes)
    nseg=4,           # x segments for int16 gather indices
    nblk=52,          # main (unique-destination) blocks (~24 chunks each;
                      # >~40-chunk blocks overflow the SWDGE ring and hang)
    nbuf=3,           # message buffers in flight
)


def host_prep(x, edge_index, params=DEFAULT_PARAMS):
    """Deal each destination row's edges across distinct blocks (uniqueness
    within a block), group by source segment within a block, pad to 128-token
    chunks. All cores share one program: per-(block, seg) chunk counts are
    maxed over cores. Returns (per_core_inputs, T, blocks, out_rows, trash)."""
    p = params
    ncores, shard, nseg, nblk = p["ncores"], p["shard"], p["nseg"], p["nblk"]
    segsz = p["n_nodes"] // nseg
    assert nseg * segsz == p["n_nodes"] and ncores * shard == p["n_nodes"]
    trash = shard + (-shard) % 128
    out_rows = trash + 128

    row = np.asarray(edge_index[0], dtype=np.int64)
    col = np.asarray(edge_index[1], dtype=np.int64)

    # ---- per-core edge lists with block assignment ----
    core_of = row // shard
    per_core_edges = []   # (blk, seg, c_loc, r_loc) arrays
    max_k = 0
    for k in range(ncores):
        m = core_of == k
        r = (row[m] - k * shard).astype(np.int64)
        c = col[m]
        order = np.argsort(r, kind="stable")
        r, c = r[order], c[order]
        # rank of each edge within its row group: 0..k_r-1
        grp_start = np.r_[0, np.nonzero(np.diff(r))[0] + 1]
        counts = np.diff(np.r_[grp_start, len(r)])
        max_k = max(max_k, int(counts.max()) if len(counts) else 0)
        rank = np.arange(len(r)) - np.repeat(grp_start, counts)
        # pseudo-random per-row start offset for balance
        h = (r * 2654435761) % nblk
        blk = (np.repeat(h[grp_start], counts) + rank)  # rank < nblk -> main
        seg = c // segsz
        per_core_edges.append((blk, rank, seg,
                               (c - seg * segsz).astype(np.int16),
                               r.astype(np.int16)))

    n_clean = max(2, max_k - nblk)   # cleanup blocks for spilled ranks
    nblk_tot = nblk + n_clean

    # resolve final block id (main: (h+rank) % nblk ; spill: nblk + (rank-nblk))
    counts_bs = np.zeros((ncores, nblk_tot, nseg), dtype=np.int64)
    resolved = []
    for k in range(ncores):
        blk, rank, seg, c_loc, r_loc = per_core_edges[k]
        main = rank < nblk
        b = np.where(main, blk % nblk, nblk + (rank - nblk))
        assert b.max(initial=0) < nblk_tot
        np.add.at(counts_bs[k], (b, seg), 1)
        resolved.append((b, seg, c_loc, r_loc))

    # per-(block, seg) chunk counts, shared across cores
    chunks_bs = -(np.max(counts_bs, axis=0) // -128)   # [nblk_tot, nseg]
    tok_bs = chunks_bs * 128
    # token offset of each (block, seg) group in the global stream
    off_bs = np.zeros_like(tok_bs)
    off = 0
    blocks = []   # per block: (tok0, ntok, [(seg, sub_tok0, nchunks), ...])
    for b in range(nblk_tot):
        tok0 = off
        subs = []
        for s in range(nseg):
            off_bs[b, s] = off
            if chunks_bs[b, s] > 0:
                subs.append((s, off, int(chunks_bs[b, s])))
            off += int(tok_bs[b, s])
        ntok = off - tok0
        if ntok > 0:
            blocks.append((tok0, ntok, subs))
    T = off
    assert T % 128 == 0

    per_core = []
    x = np.asarray(x, dtype=np.float32)
    for k in range(ncores):
        b, seg, c_loc, r_loc = resolved[k]
        gidx = np.zeros(T, dtype=np.int16)          # pad gathers read x_seg[0]
        sidx = np.full(T, trash, dtype=np.int16)    # pad scatters hit trash row
        # position within each (b, seg) cell
        order = np.lexsort((seg, b))
        bs_sorted = b[order] * nseg + seg[order]
        starts = np.r_[0, np.nonzero(np.diff(bs_sorted))[0] + 1]
        cnts = np.diff(np.r_[starts, len(bs_sorted)])
        within = np.arange(len(bs_sorted)) - np.repeat(starts, cnts)
        tok = off_bs[b[order], seg[order]] + within
        gidx[tok] = c_loc[order]
        sidx[tok] = r_loc[order]
        gw = np.tile(gidx.reshape(-1, 16).T, (8, 1)).copy()
        sw = np.tile(sidx.reshape(-1, 16).T, (8, 1)).copy()
        per_core.append({"x": x, "gidx": gw, "sidx": sw})

    return per_core, T, blocks, out_rows, trash


def build_bass(T, blocks, params=DEFAULT_PARAMS, out_rows=None):
    import concourse.bacc as bacc
    import concourse.mybir as mybir
    import contextlib

    p = params
    d, nseg, nbuf = p["d"], p["nseg"], p["nbuf"]
    segsz = p["n_nodes"] // nseg

    nc = bacc.Bacc(
        None, target_bir_lowering=False, debug=False, num_swdge_queues=2
    )
    x = nc.dram_tensor("x", [p["n_nodes"], d], mybir.dt.float32, kind="ExternalInput")
    gidx = nc.dram_tensor("gidx", [128, T // 16], mybir.dt.int16, kind="ExternalInput")
    sidx = nc.dram_tensor("sidx", [128, T // 16], mybir.dt.int16, kind="ExternalInput")
    out = nc.dram_tensor("out", [out_rows, d], mybir.dt.float32, kind="ExternalOutput")

    NB = len(blocks)
    max_chunks = max(ntok for _, ntok, _ in blocks) // 128
    # cap tokens per DMA call so its descriptor stream fits the SWDGE ring
    # (~256 descs per engine lane; scatter tx pushes ~ntok/8 per lane)
    cap_ch = 15
    # cumulative sub-gather count per buffer slot, for exact gsem waits
    gcnt = [0] * nbuf
    scnt = [0]  # cumulative scatter call count

    with (
        nc.sbuf_tensor([128, T // 16], mybir.dt.int16) as gi_sb,
        nc.sbuf_tensor([128, T // 16], mybir.dt.int16) as si_sb,
        nc.sbuf_tensor([128, nbuf * max_chunks * d], mybir.dt.float32) as msg,
        nc.semaphore("lsem") as lsem,
        nc.semaphore("ssem") as ssem,
        contextlib.ExitStack() as stack,
        nc.Block() as block,
    ):
        gsems = [stack.enter_context(nc.semaphore(f"gsem{i}")) for i in range(nbuf)]

        @block.gpsimd
        def _(g):
            g.dma_start(out=gi_sb[:], in_=gidx[:]).then_inc(lsem, 16)
            g.dma_start(out=si_sb[:], in_=sidx[:]).then_inc(lsem, 16)
            g.wait_ge(lsem, 32)

            def gathers(j):
                tok0, ntok, subs = blocks[j]
                i = j % nbuf
                base = i * max_chunks * d
                for s, sub0, nch in subs:
                    for c0 in range(0, nch, cap_ch):
                        cc = min(cap_ch, nch - c0)
                        p0 = sub0 + c0 * 128
                        boff = base + ((sub0 - tok0) // 128 + c0) * d
                        buf = msg[:, boff:boff + cc * d]
                        g.dma_gather(
                            out_ap=buf.rearrange("p (k dd) -> p k dd", dd=d),
                            in_ap=x[s * segsz:(s + 1) * segsz, :],
                            idxs_ap=gi_sb[:, p0 // 16:(p0 + cc * 128) // 16],
                            num_idxs=cc * 128,
                            num_idxs_reg=cc * 128,
                            elem_size=d,
                            queue_num=1,
                        ).then_inc(gsems[i], 16)
                        gcnt[i] += 1

            def scatter(b):
                tok0, ntok, _ = blocks[b]
                i = b % nbuf
                base = i * max_chunks * d
                g.wait_ge(gsems[i], 16 * gcnt[i])   # all sub-gathers of block b
                nch = ntok // 128
                for c0 in range(0, nch, cap_ch):
                    cc = min(cap_ch, nch - c0)
                    p0 = tok0 + c0 * 128
                    g.dma_scatter_add(
                        out_ap=out[:],
                        in_ap=msg[:, base + c0 * d:base + (c0 + cc) * d].rearrange(
                            "p (k dd) -> p k dd", dd=d),
                        idxs_ap=si_sb[:, p0 // 16:(p0 + cc * 128) // 16],
                        num_idxs=cc * 128,
                        num_idxs_reg=cc * 128,
                        elem_size=d,
                        queue_num=0,
                    ).then_inc(ssem, 16)
                    scnt[0] += 1

            for j in range(min(nbuf - 1, NB)):
                gathers(j)
            for b in range(NB):
                # serialize between blocks: all of block b-1's scatters fully
                # landed (also frees the buffer slot gathers b+nbuf-1 reuse)
                if b > 0:
                    g.wait_ge(ssem, 16 * scnt[0])
                jg = b + nbuf - 1
                if jg < NB:
                    gathers(jg)
                scatter(b)
            g.wait_ge(ssem, 16 * scnt[0])

    nc.compile()
    return nc


def run_spmd(nc, per_core, trace=False):
    from concourse.bass_utils import run_bass_kernel_spmd
    return run_bass_kernel_spmd(
        nc, per_core, core_ids=list(range(len(per_core))), trace=trace
    )


def kernel(x, edge_index, _trace=False, _return_results=False):
    x = np.asarray(x, dtype=np.float32)
    params = DEFAULT_PARAMS
    per_core, T, blocks, out_rows, trash = host_prep(x, edge_index, params)
    nc = build_bass(T, blocks, params, out_rows)
    res = run_spmd(nc, per_core, trace=_trace)
    shard = params["shard"]
    out = np.concatenate(
        [res.results[k]["out"][:shard] for k in range(params["ncores"])], axis=0)
    if _return_results:
        return out, res
    return out
